# revision 33
# baseline (speedup 1.0000x reference)
"""BinaryLinear forward on 8 Trainium2 NeuronCores.

Computes out = x @ sign(weight).T for x:[16384,2048] (values in {-1,+1}),
weight:[2048,2048] -> out:[16384,2048] fp32 — bit-exact vs the fp32
reference.

Strategy (data-parallel per the sharding hint): shard x rows across the 8
cores (2048 each), replicate the binarized weight. Both operands are
exactly +/-1, so they are cast to fp8e4 (exact) and the matmul runs in
DoubleRow perf mode (2 fp8 weights per PE cell -> K=256 per matmul, 2x
bf16 throughput) accumulating in fp32 PSUM; sums are even integers
<= 2048, exact in fp32 and also in the fp16 used for the output DMA
(halved write traffic), upcast to fp32 on the host.

Kernel layout/scheduling notes (v4):
 - x and w are pre-transposed on the host so K lands on the SBUF
   partition dim; both stay SBUF-resident (4.2 MB each per core).
 - the startup-critical first phase (x mh0 / w j0, 2 MB) is HOST-PACKED
   into kc-pair chunks whose DRAM rows are 2KB contiguous (faster HWDGE
   lines); ALL first-phase chunks are per-kc 128KB tiles DMA'd in exact
   consumption order, alternating x-on-sync / w-on-scalar, with the kc0
   w-chunk ks-split across BOTH rings (64KB each) so the very first
   matmul group gates on two small parallel transfers (~10.4us). Per-kc
   tiles keep the subtile dependency tracking exact (odd slices get
   conservatively widened to whole tiles -- v3 lost 1us to that), and
   the early-DMA count stays low because each DMA_DIRECT2D costs ~0.7us
   of engine issue time.
 - 14 DoubleRow + 9 plain N=256 dummy warmup matmuls bridge the PE
   from preamble-end (~7.3us) through the first-data jitter band. The
   HAM clock gate needs an uninterrupted 4096-cycle busy window and
   any PE idle gap before entry resets it (costing ~2.5us of
   half-clock matmuls), so dummies must cover D_max; the plain-mode
   tail dummies shrink to ~107ns each once the clock is warm, keeping
   the blocking cost of overshoot small.
 - all outputs are STAGED IN SBUF (per-group [P, gsz, NB] fp16 tiles,
   ~8MB total, no buffer reuse -> casts never wait on output DMAs) and
   leave through the two HWDGE rings (sync/scalar), queued BEHIND the
   input descriptors of the same rings. Mid-kernel output transfers
   only need to finish by kernel end, and the rings drain inputs by
   ~45us, outputs by ~80us. gpsimd issues NO DMAs at all: the v1
   SWDGE path cost a 6.2us gpsimd DRAIN on the exit critical path
   (~100ns per software-DGE descriptor). The last o-phase DMAs per-mo
   (128KB) as each cast lands so the rings carry no backlog at kernel
   end, and the very last mo-block accumulates h-major into two N=256
   psums (all kc for h0, then h1): h0's cast+64KB DMA overlap h1's
   matmuls and only h1's cast (0.41us) + two parallel 32KB DMAs trail
   the final matmul.
Fixed overheads measured on this runtime (count toward HW exec time):
NRT preamble tail ~1.5us, NRT postamble ~8us (51 serialized semaphore
resets per engine; PE's NX is slowest at ~133ns each), ~2.2us of
periodic 432ns matmul-slot stalls every 10.8us (profiling tax), PE
stream floor 110.5us (fp8 DoubleRow, 504x N=512 + 16x N=256 at
~216ns/108ns). Measured: v1 131.2us, v2 128.7us, this version (v4)
127.4us. Variants that started the matmul stream 0.7-1us earlier
(combined first-block / 2KB-line pair chunks / trimmed dummies)
measured 128.0-128.6us: the first phase is HWDGE-ring-bandwidth-bound
(~120-140GB/s/ring early), so any earlier start converts head time
into kc2+ supply stalls, plus a larger half-clock window before the
HAM clock-gate warms. This version's later start keeps the stream
gap-free, which measured best across 8 hardware runs of 6 variants.
CAUTION for future edits: slice tiles DIRECTLY in one expression;
storing a dim-dropped AP view and re-slicing it later silently loses
the DMA-completion dependency (intermittent wrong results, measured).
Occasional ~154us runs are a chip-level power-throttle state
(throttle_avg_util_limit 87.5% in the NTFF), not kernel-dependent.
"""

import numpy as np
import ml_dtypes

import concourse.mybir as mybir
import concourse.tile as tile
from concourse import bacc
from concourse.bass_utils import run_bass_kernel_spmd

M, K, O = 16384, 2048, 2048
N_CORES = 8
MS = M // N_CORES
P = 128
KO2 = K // (2 * P)         # 8 double-row k-chunks
NB = 512
NJ = O // NB               # 4 o-blocks
MO = MS // P               # 16 m-blocks
MH = 4                     # m-blocks per phase
MB = MH * P                # 512 m-cols per phase
NMH = MO // MH             # 4 m-phases per o-block

FP8 = mybir.dt.float8e4

_CACHE = {}


def _build():
    if "nc" in _CACHE:
        return _CACHE["nc"]

    nc = bacc.Bacc("TRN2", target_bir_lowering=False, debug=False,
                   num_devices=N_CORES)
    xT = nc.dram_tensor("xT", [K, MS], FP8, kind="ExternalInput")
    wT = nc.dram_tensor("wT", [K, O], FP8, kind="ExternalInput")
    # Host-packed first-phase blocks (x mh0 / w j0) in kc-pair chunks whose
    # DRAM rows are 2KB contiguous (vs the 512B runs the [K, MS] layout
    # gives) -- bigger lines lift the per-queue HWDGE byte rate on the
    # startup-critical first 2MB.
    xF = nc.dram_tensor("xF", [4 * P, 2 * 2 * MB], FP8, kind="ExternalInput")
    wF = nc.dram_tensor("wF", [4 * P, 2 * 2 * NB], FP8, kind="ExternalInput")
    out = nc.dram_tensor("out", [MS, O], mybir.dt.float16,
                         kind="ExternalOutput")

    xT_v = xT.ap().rearrange("(kc ks pi) m -> pi kc ks m", pi=P, ks=2)
    wT_v = wT.ap().rearrange("(kc ks pi) o -> pi kc ks o", pi=P, ks=2)
    xF_v = xF.ap().rearrange("(c pi) (kc2 ks m) -> c pi kc2 ks m",
                             pi=P, kc2=2, ks=2)
    wF_v = wF.ap().rearrange("(c pi) (kc2 ks o) -> c pi kc2 ks o",
                             pi=P, kc2=2, ks=2)
    out_v = out.ap().rearrange("(mo pi) o -> pi mo o", pi=P)

    with tile.TileContext(nc) as tc:
        with tc.tile_pool(name="xres", bufs=1) as x_pool, \
             tc.tile_pool(name="wres", bufs=1) as w_pool, \
             tc.tile_pool(name="outs", bufs=1) as out_pool, \
             tc.tile_pool(name="psum", bufs=8, space="PSUM") as psum_pool:

            xfk = [None] * KO2      # mh0 per-kc chunk: [P, 2, MB]
            wfk = [None] * KO2      # j0 per-kc chunk: [P, 2, NB]
            x_t = [[None] * NMH for _ in range(KO2)]   # mh1..3 chunks
            w_t = [[None] * NJ for _ in range(KO2)]    # j1..3 chunks
            alt = [0]

            def _eng():
                alt[0] += 1
                return nc.sync if alt[0] % 2 == 0 else nc.scalar

            def alloc_xfk(kc):
                t = x_pool.tile([P, 2, MB], FP8, tag=f"xfk{kc}",
                                name=f"xfk{kc}")
                xfk[kc] = t
                return t

            def alloc_wfk(kc):
                t = w_pool.tile([P, 2, NB], FP8, tag=f"wfk{kc}",
                                name=f"wfk{kc}")
                wfk[kc] = t
                return t

            def load_x(kc, mq):
                t = x_pool.tile([P, 2, MB], FP8, tag=f"x{kc}_{mq}",
                                name=f"x{kc}_{mq}")
                _eng().dma_start(t[:], xT_v[:, kc, :, mq * MB:(mq + 1) * MB])
                x_t[kc][mq] = t

            def load_w(kc, j):
                t = w_pool.tile([P, 2, NB], FP8, tag=f"w{kc}_{j}",
                                name=f"w{kc}_{j}")
                _eng().dma_start(t[:], wT_v[:, kc, :, j * NB:(j + 1) * NB])
                w_t[kc][j] = t

            # PE warmup: dummy matmuls keep the PE busy from preamble-end
            # (~7.3us) until first data. The HAM busy window (running since
            # preamble-end) needs NO PE-idle gap before it completes, else it
            # resets and the stream runs at 1.2GHz for another 3.4-6.8us
            # (~2.5us penalty). 15 N=256 DoubleRow dummies (~213ns cold)
            # reach ~10.5us = the p10 of first-data arrival; 12 plain N=256
            # dummies (~213ns cold / ~107ns warm) then cover the jitter band
            # to ~11.8-13us at finer blocking granularity, so a late chunk
            # never sees a gap and an early chunk is blocked <= one dummy.
            # Raw (non-pool) SBUF tensor: no writer needed, so the PE can
            # issue dummies the moment its preamble ends. Garbage operand
            # values are fine: the PSUM result is never read.
            zw = nc.alloc_sbuf_tensor("zwarm_raw", [P, 2, 2 * P], FP8).ap()
            pz = psum_pool.tile([P, 2 * P], mybir.dt.float32, tag="ps",
                                name="ps_warm")
            for _ in range(14):
                nc.tensor.matmul(
                    pz[:], zw[:, :, 0:P], zw[:],
                    start=True, stop=True,
                    perf_mode=mybir.MatmulPerfMode.DoubleRow,
                )
            for _ in range(9):
                nc.tensor.matmul(pz[:], zw[:, 0, 0:P], zw[:, 0],
                                 start=True, stop=True)

            # Emission (= per-queue arrival) order mirrors consumption order;
            # all first-phase chunks are 128KB per-kc tiles so every
            # consumer gates on exactly one (or for wfk0, two) writer DMAs
            # and never on a later chunk (the subtile dep tracker widens
            # odd slices to whole tiles -- per-kc tiles make that exact).
            # The kc0 w-chunk is ks-split ACROSS BOTH rings (64KB each) so
            # the very first matmul group gates on two small parallel
            # transfers; each ring then alternates so neither is more than
            # one chunk ahead of consumption order. Per-DMA engine issue
            # costs ~0.7us, so the count of early DMAs is kept low.
            # Output DMAs are emitted later, so they land BEHIND all inputs
            # in the same two HWDGE rings.
            t = alloc_xfk(0)
            nc.sync.dma_start(t[:], xF_v[0][:, 0])
            t = alloc_wfk(0)
            nc.scalar.dma_start(t[:, 1], wF_v[0][:, 0, 1])
            nc.sync.dma_start(t[:, 0], wF_v[0][:, 0, 0])
            t = alloc_xfk(1)
            nc.scalar.dma_start(t[:], xF_v[0][:, 1])
            t = alloc_wfk(1)
            nc.sync.dma_start(t[:], wF_v[0][:, 1])
            # kc2/kc3 ride the gpsimd SWDGE as a THIRD ring: the two HWDGE
            # rings are bandwidth-marginal through kc3 (~130GB/s each while
            # ramping), and these two chunks were the only measured
            # mid-stream supply stalls (0.43+0.37us; 1.3us when an early
            # HAM draw started the stream warm). SWDGE starts ~2-3us after
            # issue but then moves 512KB well before the ~13us deadline,
            # and unloading it lets kc4+ on the HWDGE rings gain margin.
            # Only 4 SWDGE descriptors, all complete by ~13us, so the exit
            # gpsimd DRAIN stays short and overlaps Sync's completion waits.
            for kc in (2, 3):
                c, kci = divmod(kc, 2)
                t = alloc_xfk(kc)
                nc.gpsimd.dma_start(t[:], xF_v[c][:, kci])
                t = alloc_wfk(kc)
                nc.gpsimd.dma_start(t[:], wF_v[c][:, kci])
            for kc in range(4, KO2):
                c, kci = divmod(kc, 2)
                t = alloc_xfk(kc)
                nc.sync.dma_start(t[:], xF_v[c][:, kci])
                t = alloc_wfk(kc)
                nc.scalar.dma_start(t[:], wF_v[c][:, kci])
            for mq in range(1, NMH):
                for kc in range(KO2):
                    load_x(kc, mq)
            for j in range(1, NJ):
                for kc in range(KO2):
                    load_w(kc, j)

            # Final phase-groups shrink to size 1 so the end-of-kernel cast+
            # DMA tail is shorter (fewer serialized PSUM evictions after the
            # very last matmul).
            full = [(s, MH) for s in range(0, MO, MH)]
            tail_split = full[:-1] + [(MO - MH, 2), (MO - 2, 1), (MO - 1, 1)]
            onum = [0]

            def _oeng():
                onum[0] += 1
                return nc.sync if onum[0] % 2 == 1 else nc.scalar

            for j in range(NJ):
                groups = tail_split if j == NJ - 1 else full
                for gi, (mo0, gsz) in enumerate(groups):
                    if j == NJ - 1 and gi == len(groups) - 1:
                        # Very last mo-block: two N=256 psums, accumulated
                        # h-MAJOR (all kc for h0, then h1) so h0's cast and
                        # 64KB DMA overlap h1's matmuls; after the final
                        # matmul only h1's cast (0.41us) + two parallel 32KB
                        # DMAs remain on the exit critical path.
                        mo = mo0
                        mr = mo % MH
                        psH = [psum_pool.tile([P, NB // 2], mybir.dt.float32,
                                              tag="ps", name=f"ps_f{h}")
                               for h in range(2)]
                        ots = []
                        for h in range(2):
                            for kc in range(KO2):
                                xa = x_t[kc][mo // MH][:, :,
                                                       mr * P:(mr + 1) * P]
                                wa = w_t[kc][j][:]
                                nc.tensor.matmul(
                                    psH[h][:], xa,
                                    wa[:, :,
                                       h * (NB // 2):(h + 1) * (NB // 2)],
                                    start=(kc == 0),
                                    stop=(kc == KO2 - 1),
                                    perf_mode=mybir.MatmulPerfMode.DoubleRow,
                                )
                            ot = out_pool.tile([P, NB // 2], mybir.dt.float16,
                                               tag=f"ot_f{h}", name=f"ot_f{h}")
                            nc.vector.tensor_copy(out=ot[:], in_=psH[h][:])
                            ots.append(ot)
                        col0 = j * NB
                        h2 = NB // 2
                        q4 = NB // 4
                        # h0: one 64KB DMA on scalar, issued while h1's
                        # matmuls run (sync stays free so h1's first half
                        # below issues with zero engine wait).
                        nc.scalar.dma_start(
                            out_v[:, mo, col0:col0 + h2], ots[0][:])
                        # h1: two parallel 32KB halves on both (empty) rings.
                        nc.sync.dma_start(
                            out_v[:, mo, col0 + h2:col0 + h2 + q4],
                            ots[1][:, :q4])
                        nc.scalar.dma_start(
                            out_v[:, mo, col0 + h2 + q4:col0 + NB],
                            ots[1][:, q4:])
                        continue
                    psums = [psum_pool.tile([P, NB], mybir.dt.float32,
                                            tag="ps", name=f"ps_{j}_{gi}_{i}")
                             for i in range(gsz)]
                    for kc in range(KO2):
                        for mi in range(gsz):
                            mo = mo0 + mi
                            mh, mr = divmod(mo, MH)
                            if mh == 0:
                                xa = xfk[kc][:, :, mr * P:(mr + 1) * P]
                            else:
                                xa = x_t[kc][mh][:, :, mr * P:(mr + 1) * P]
                            if j == 0:
                                wa = wfk[kc][:]
                            else:
                                wa = w_t[kc][j][:]
                            nc.tensor.matmul(
                                psums[mi][:], xa, wa,
                                start=(kc == 0),
                                stop=(kc == KO2 - 1),
                                perf_mode=mybir.MatmulPerfMode.DoubleRow,
                            )
                    # Stage the whole group in SBUF (dedicated tile, never
                    # reused -> the casts free the psum banks immediately and
                    # never wait on an output DMA), then one batched DMA
                    # rides a HWDGE ring behind the input descriptors. In
                    # the last o-phase the batches shrink toward the end
                    # (512/256/128KB) and alternate rings, so the rings hold
                    # no backlog when the final tiny DMAs arrive, while the
                    # engines stay free of extra ~0.7us DMA-issue slots.
                    og = out_pool.tile([P, gsz, NB], mybir.dt.float16,
                                       tag=f"og{j}_{gi}", name=f"og{j}_{gi}")
                    for mi in range(gsz):
                        nc.vector.tensor_copy(out=og[:, mi], in_=psums[mi][:])
                        if j == NJ - 1 and gi == 2 and mi in (1, 3):
                            oeng = nc.sync if mi == 1 else nc.scalar
                            oeng.dma_start(
                                out_v[:, mo0 + mi - 1:mo0 + mi + 1,
                                      j * NB:(j + 1) * NB],
                                og[:, mi - 1:mi + 1])
                    if j == NJ - 1:
                        if gi == 0:
                            nc.sync.dma_start(
                                out_v[:, mo0:mo0 + gsz,
                                      j * NB:(j + 1) * NB], og[:])
                        elif gi == 1 or gi == 3:
                            nc.scalar.dma_start(
                                out_v[:, mo0:mo0 + gsz,
                                      j * NB:(j + 1) * NB], og[:])
                        elif gi == 4:
                            nc.sync.dma_start(
                                out_v[:, mo0:mo0 + gsz,
                                      j * NB:(j + 1) * NB], og[:])
                    else:
                        _oeng().dma_start(
                            out_v[:, mo0:mo0 + gsz, j * NB:(j + 1) * NB],
                            og[:])

    nc.compile()
    _CACHE["nc"] = nc
    return nc


def _build_bf16():
    """Fallback: plain bf16 matmul via the library composable kernel.

    Only used if x is ever not exactly +/-1 (outside the stated input
    contract), where the fp8 cast would be lossy. bf16 keeps the result
    within ~1e-3 relative of the fp32 reference for gaussian x.
    """
    if "nc16" in _CACHE:
        return _CACHE["nc16"]
    from concourse.kernels.tile_matmul import matmul_tile_kernel

    nc = bacc.Bacc("TRN2", target_bir_lowering=False, debug=False,
                   num_devices=N_CORES)
    xT = nc.dram_tensor("xT", [K, MS], mybir.dt.bfloat16,
                        kind="ExternalInput")
    wT = nc.dram_tensor("wT", [K, O], mybir.dt.bfloat16,
                        kind="ExternalInput")
    out = nc.dram_tensor("out", [MS, O], mybir.dt.float32,
                         kind="ExternalOutput")
    with tile.TileContext(nc) as tc:
        matmul_tile_kernel(tc, xT.ap(), wT.ap(), out.ap())
    nc.compile()
    _CACHE["nc16"] = nc
    return nc


def _binarize_weight(weight):
    # sign(sign(w) + 0.5): maps 0 -> +1, else +/-1 (matches the reference)
    return np.sign(np.sign(weight, dtype=np.float32) + np.float32(0.5))


def _pack_first(aT):
    # [K, >=512] -> kc-pair-chunked first block [4*128, 2048] whose rows are
    # 2KB contiguous: (c, p, kc2, ks, m) = aT[((c*2+kc2)*2+ks)*128 + p, m]
    r = aT[:, :512].reshape(4, 2, 2, P, 512)
    return np.ascontiguousarray(
        r.transpose(0, 3, 1, 2, 4).reshape(4 * P, 2 * 2 * 512))


def prepare_in_maps(x, weight, dtype=ml_dtypes.float8_e4m3):
    x = np.asarray(x, dtype=np.float32)
    weight = np.asarray(weight, dtype=np.float32)
    wT_h = np.ascontiguousarray(_binarize_weight(weight).T.astype(dtype))
    xT_h = np.ascontiguousarray(x.T.astype(dtype))
    if dtype != ml_dtypes.float8_e4m3:
        return [
            {"xT": np.ascontiguousarray(xT_h[:, c * MS:(c + 1) * MS]),
             "wT": wT_h}
            for c in range(N_CORES)
        ]
    wF_h = _pack_first(wT_h)
    maps = []
    for c in range(N_CORES):
        xTc = np.ascontiguousarray(xT_h[:, c * MS:(c + 1) * MS])
        maps.append({"xT": xTc, "wT": wT_h,
                     "xF": _pack_first(xTc), "wF": wF_h})
    return maps


def gather_output(results):
    return np.concatenate(
        [results[c]["out"] for c in range(N_CORES)], axis=0
    ).astype(np.float32)


def kernel(x, weight):
    x = np.asarray(x, dtype=np.float32)
    if bool(np.all(np.abs(x) == 1.0)):
        nc = _build()
        in_maps = prepare_in_maps(x, weight)
    else:
        nc = _build_bf16()
        in_maps = prepare_in_maps(x, weight, dtype=ml_dtypes.bfloat16)
    res = run_bass_kernel_spmd(nc, in_maps, core_ids=list(range(N_CORES)))
    return gather_output(res.results)
